# revision 14
# baseline (speedup 1.0000x reference)
"""MoE (top-2 of 8 experts, SwiGLU) Trainium2 kernel, expert-parallel over 8 cores.

Contract: kernel(**inputs) takes the FULL unsharded inputs
  x [2,2048,1024] f32, gate_w [8,1024] f32,
  w1 [8,2048,1024] f32, w2 [8,1024,2048] f32, w3 [8,2048,1024] f32
and returns the FULL output [2,2048,1024] f32.

Strategy (expert-parallel, per the hint "replicate the gate and all-to-all the
token dispatch"): routing (gate softmax + top-2) is computed on host; tokens
are dispatched (gathered) per expert; core e runs the SwiGLU FFN of expert e
over its ~N*TOPK/E assigned tokens (padded to capacity C), pre-scaled by the
combine weight; the host scatter-adds the two expert contributions per token.

Device kernel (per core, feature-major layout so no on-device transposes).
All matmul operands are bf16 (same 1 cycle/row PE rate as fp32r at 512-wide
moving dim, but half the DMA bytes, half the LDWEIGHTS time so weight loads
fully hide under the matmul stream, and half the SBUF footprint; fp32 PSUM
accumulate keeps rel err ~4e-3):
  h1T = w1 @ xg^T   [H, C]   (lhsT = w1T block, rhs = xgT)
  h3T = w3 @ xg^T   [H, C]
  aT  = silu(h1T) * h3T      (ACT Silu + DVE mul, PSUM->SBUF, bf16)
  yT  = (w2 @ aT) * combine  [D, C]  (DVE mul on PSUM eviction, f32 out)

Tokens are processed in free-dim chunks of <=512 (PSUM-bank bound). DMA is
spread over the three DGE queues (sync/scalar HWDGE + gpsimd SWDGE) so the
first chunk of xg and the first h-block's weights arrive in parallel, which
sets the time-to-first-matmul.
"""

import math
import sys

import numpy as np

for _p in ("/opt/trn_rl_repo", "/opt/pypackages"):
    if _p not in sys.path:
        sys.path.append(_p)

import ml_dtypes  # noqa: E402

import concourse.bass as bass  # noqa: E402
import concourse.tile as tile  # noqa: E402
from concourse import bacc, mybir  # noqa: E402
from concourse.bass_utils import run_bass_kernel_spmd  # noqa: E402

B, T, D, H, E, TOPK = 2, 2048, 1024, 2048, 8, 2
N = B * T
P = 128
KD = D // P   # 8  k-tiles over D
KD2 = KD // 2  # 4 k-tiles per xg half tile
KH = H // P   # 16 k-tiles over H
HB = H // P   # 16 h blocks of 128 (M dim, stage A)
DB = D // P   # 8  d blocks of 128 (M dim, stage B)

F32 = mybir.dt.float32
BF16 = mybir.dt.bfloat16
NP_BF16 = ml_dtypes.bfloat16

# set by test.py to capture an NTFF profile; kernel() stores results here
TRACE = False
TRACE_ALL_CORES = False
LAST_RESULTS = None

_program_cache = {}

# CoreSim doesn't implement Silu; simcheck.py overrides this to Sigmoid.
_ACT_FUNC = mybir.ActivationFunctionType.Silu


# Max tokens per expert handled on host when the count barely exceeds a
# 512 multiple (capacity-factor overflow): full 512-wide chunks minimize the
# per-matmul dispatch overhead.
OVERFLOW_MAX = 64


def _chunk_plan(cmax: int) -> list[int]:
    """Token-chunk sizes for the device capacity: each <=512 (PSUM bank),
    as equal as possible, 32-aligned, minimal total padding. If cmax is
    within OVERFLOW_MAX above a 512 multiple, use full 512 chunks and let
    the caller route the overflow tokens to the host FFN."""
    if cmax >= 512 and cmax - (cmax // 512) * 512 <= OVERFLOW_MAX:
        return [512] * (cmax // 512)
    n = max(1, math.ceil(cmax / 512))
    chunks = []
    rem = cmax
    for i in range(n):
        s = math.ceil(rem / (n - i) / 32) * 32
        s = min(max(s, 256), 512)
        chunks.append(s)
        rem -= s
    return chunks


def _host_ffn(x_rows, w1e, w2e, w3e, wts):
    """Host-side SwiGLU FFN for capacity-overflow tokens (<=64/expert),
    in bf16 operand precision to match the device kernel."""
    xb = x_rows.astype(NP_BF16).astype(np.float32)
    h1 = xb @ w1e.astype(NP_BF16).astype(np.float32).T
    h3 = xb @ w3e.astype(NP_BF16).astype(np.float32).T
    a = h1 / (1.0 + np.exp(-h1)) * h3
    a = a.astype(NP_BF16).astype(np.float32)
    return (a @ w2e.astype(NP_BF16).astype(np.float32).T) * wts[:, None]


def _build_program(chunks: list[int]):
    """Bass program for one core: expert FFN over C = sum(chunks) tokens."""
    C = sum(chunks)
    offs = [sum(chunks[:i]) for i in range(len(chunks))]
    tsls = [bass.ds(o, s) for o, s in zip(offs, chunks)]
    nt = len(chunks)

    nc = bacc.Bacc(
        "TRN2", target_bir_lowering=False, debug=False,
        enable_asserts=False, num_devices=8,
    )
    # All inputs are pre-tiled on host into partition-major layouts so every
    # DMA reads one big contiguous run per partition (2-8 KB descriptors).
    # Feature-split [D,C]-style layouts would give 256B-1KB runs in bf16,
    # which leaves the packet-rate-limited DMA engines at a fraction of the
    # ~360 GB/s per-core HBM bandwidth.
    #   xgLo/xgHi[p, t-block]: per chunk t a [KD2, chunk] block, flattened
    #   w1L/w3L[hb, p, :]: h-block hb's stationary tile [KD, P], flattened
    #   w2L[db, p, :]: d-block db's stationary tile [KH, P], flattened
    xgLo_d = nc.dram_tensor("xgLo", [P, KD2 * C], BF16,
                            kind="ExternalInput").ap()
    xgHi_d = nc.dram_tensor("xgHi", [P, KD2 * C], BF16,
                            kind="ExternalInput").ap()
    w1L_d = nc.dram_tensor("w1L", [HB, P, KD * P], BF16,
                           kind="ExternalInput").ap()
    w3L_d = nc.dram_tensor("w3L", [HB, P, KD * P], BF16,
                           kind="ExternalInput").ap()
    w2L_d = nc.dram_tensor("w2L", [DB, P, KH * P], BF16,
                           kind="ExternalInput").ap()
    scl_d = nc.dram_tensor("scale_b", [P, C], F32, kind="ExternalInput").ap()
    yT_d = nc.dram_tensor("yT", [D, C], F32, kind="ExternalOutput").ap()

    def xg_src(dram, t):
        return dram[:, KD2 * offs[t]:KD2 * (offs[t] + chunks[t])]

    with tile.TileContext(nc) as tc:
        with tc.tile_pool(name="resident", bufs=1) as res_pool, \
             tc.tile_pool(name="w13", bufs=3) as w13_pool, \
             tc.tile_pool(name="w2", bufs=3) as w2_pool, \
             tc.tile_pool(name="ev", bufs=3) as ev_pool, \
             tc.tile_pool(name="psum", bufs=2, space="PSUM") as ps_pool:

            # PE warm-up: the Tensor clock needs ~3 us of continuous busy to
            # reach 2.4 GHz (it idles at 0.65 GHz and a >0.1 us gap drops it
            # to 1.2 GHz). Dummy matmuls on a memset tile burn the initial
            # DMA wait so the first real matmul runs at full clock. The
            # dummies accumulate into one never-read psum tile on the "y"
            # tag (stage B's ring, unused during startup).
            warm = res_pool.tile([P, 3 * P], BF16, tag="warm")
            nc.vector.memset(warm[:], 0.0)
            warm_ps = ps_pool.tile([P, max(chunks)], F32, tag="y",
                                   name="warm_ps")
            NWARM = 20
            for i in range(NWARM):
                nc.tensor.matmul(warm_ps[:, 0:2 * P], warm[:, 0:P],
                                 warm[:, P:3 * P],
                                 start=(i == 0), stop=(i == NWARM - 1))

            # Startup critical path, ordered so each dependency lands
            # just-in-time for a gapless ramp (each tile is written by ONE
            # queue only; in-queue order = arrival order):
            #   sync   (HWDGE): w1t_h0, xg_lo c0, xg_lo c1, w3 h2.., w2
            #   scalar (HWDGE): xg_hi c0, w3t_h0, xg_hi c1, yT out
            #   gpsimd (SWDGE): w1t_h1, w3t_h1, w1 h2.., scl
            w1ts0, w3ts0 = [], []
            w1t = w13_pool.tile([P, KD, P], BF16, tag="w1_0",
                                bufs=2, name="w1t_0")
            nc.sync.dma_start(w1t[:], w1L_d[0])
            w1ts0.append(w1t)
            xg_lo = res_pool.tile([P, KD2, C], BF16, tag="xg_lo")
            xg_hi = res_pool.tile([P, KD2, C], BF16, tag="xg_hi")
            nc.sync.dma_start(xg_lo[:, :, tsls[0]], xg_src(xgLo_d, 0))
            nc.scalar.dma_start(xg_hi[:, :, tsls[0]], xg_src(xgHi_d, 0))
            w3t = w13_pool.tile([P, KD, P], BF16, tag="w3_0",
                                bufs=2, name="w3t_0")
            nc.scalar.dma_start(w3t[:], w3L_d[0])
            w3ts0.append(w3t)
            w1t = w13_pool.tile([P, KD, P], BF16, tag="w1_1",
                                bufs=2, name="w1t_1")
            nc.gpsimd.dma_start(w1t[:], w1L_d[1])
            w1ts0.append(w1t)
            w3t = w13_pool.tile([P, KD, P], BF16, tag="w3_1",
                                bufs=2, name="w3t_1")
            nc.gpsimd.dma_start(w3t[:], w3L_d[1])
            w3ts0.append(w3t)
            for t in range(1, nt):
                nc.sync.dma_start(xg_lo[:, :, tsls[t]], xg_src(xgLo_d, t))
                nc.scalar.dma_start(xg_hi[:, :, tsls[t]], xg_src(xgHi_d, t))
            act = res_pool.tile([P, KH, C], BF16, tag="act")

            # ---- stage A: act[H, C] = silu(w1 @ xgT) * (w3 @ xgT) ----
            # h-blocks processed in pairs with the token-chunk loop outside
            # the pair: two h-blocks of chunk-t compute run before chunk t+1
            # is touched, hiding the next xg chunk's DMA arrival. The first
            # pair's w1+w3 go on gpsimd (independent of the xg queues);
            # later pairs stream w1 on gpsimd and w3 on sync (in-order
            # behind xg, which has fully issued by then).
            for hp in range(0, HB, 2):
                pair = [h for h in (hp, hp + 1) if h < HB]
                if hp == 0:
                    w1ts = w1ts0
                    w3ts = w3ts0
                else:
                    w1ts, w3ts = [], []
                    for i, h in enumerate(pair):
                        w1t = w13_pool.tile([P, KD, P], BF16, tag=f"w1_{i}",
                                            bufs=2, name=f"w1t_{h}")
                        nc.gpsimd.dma_start(w1t[:], w1L_d[h])
                        w3t = w13_pool.tile([P, KD, P], BF16, tag=f"w3_{i}",
                                            bufs=2, name=f"w3t_{h}")
                        nc.sync.dma_start(w3t[:], w3L_d[h])
                        w1ts.append(w1t)
                        w3ts.append(w3t)
                for t in range(nt):
                    tsl = tsls[t]
                    for i, h in enumerate(pair):
                        ph1 = ps_pool.tile([P, chunks[t]], F32, tag="h1",
                                           bufs=3, name=f"ph1_{h}_{t}")
                        for k in range(KD2):
                            nc.tensor.matmul(ph1[:], w1ts[i][:, k, :],
                                             xg_lo[:, k, tsl],
                                             start=(k == 0), stop=False)
                        for k in range(KD2):
                            nc.tensor.matmul(ph1[:], w1ts[i][:, KD2 + k, :],
                                             xg_hi[:, k, tsl],
                                             start=False, stop=(k == KD2 - 1))
                        ph3 = ps_pool.tile([P, chunks[t]], F32, tag="h3",
                                           bufs=3, name=f"ph3_{h}_{t}")
                        for k in range(KD2):
                            nc.tensor.matmul(ph3[:], w3ts[i][:, k, :],
                                             xg_lo[:, k, tsl],
                                             start=(k == 0), stop=False)
                        for k in range(KD2):
                            nc.tensor.matmul(ph3[:], w3ts[i][:, KD2 + k, :],
                                             xg_hi[:, k, tsl],
                                             start=False, stop=(k == KD2 - 1))
                        asl = act[:, h, tsl]
                        nc.scalar.activation(asl, ph1[:], func=_ACT_FUNC)
                        nc.vector.tensor_mul(asl, asl, ph3[:])

            # combine-weight row (needed only for stage B evictions)
            scl = res_pool.tile([P, C], F32, tag="scl")
            nc.gpsimd.dma_start(scl[:], scl_d[:, :])

            # ---- stage B: yT[D, C] = (w2 @ act) * scale ----
            # The very last (d, t) group is split into two half-width psum
            # groups so the final evict+DMA chain after the last matmul
            # covers half the columns (the first half's eviction overlaps
            # the second half's matmuls).
            for d in range(DB):
                w2t = w2_pool.tile([P, KH, P], BF16, tag="w2")
                nc.sync.dma_start(w2t[:], w2L_d[d])
                for t in range(nt):
                    last = (d == DB - 1) and (t == nt - 1)
                    cw = chunks[t]
                    if last and cw % 2 == 0:
                        halves = [bass.ds(offs[t], cw // 2),
                                  bass.ds(offs[t] + cw // 2, cw // 2)]
                    else:
                        halves = [tsls[t]]
                    for hsl in halves:
                        hw = hsl.size
                        py = ps_pool.tile([P, hw], F32, tag="y")
                        for k in range(KH):
                            nc.tensor.matmul(py[:], w2t[:, k, :],
                                             act[:, k, hsl],
                                             start=(k == 0), stop=(k == KH - 1))
                        ysb = ev_pool.tile([P, hw], F32, tag="ysb")
                        nc.vector.tensor_mul(ysb[:], py[:], scl[:, hsl])
                        nc.scalar.dma_start(yT_d[d * P:(d + 1) * P, hsl],
                                            ysb[:])

    nc.compile()
    return nc


def _route(flat, gate_w):
    """Host replica of the reference router. Returns top-2 expert ids and
    combine weights (top-2 of softmax, renormalized)."""
    logits = flat @ gate_w.T                                   # [N, E] f32
    m = logits.max(axis=1, keepdims=True)
    p = np.exp((logits - m).astype(np.float32))
    probs = p / p.sum(axis=1, keepdims=True)
    idx = np.argsort(-probs, axis=1, kind="stable")[:, :TOPK]  # [N, 2]
    top = np.take_along_axis(probs, idx, axis=1)               # [N, 2]
    wn = top / top.sum(axis=1, keepdims=True)
    return idx, wn


def kernel(x, gate_w, w1, w2, w3):
    global LAST_RESULTS
    x = np.asarray(x, np.float32)
    gate_w = np.asarray(gate_w, np.float32)
    w1 = np.asarray(w1, np.float32)
    w2 = np.asarray(w2, np.float32)
    w3 = np.asarray(w3, np.float32)

    flat = x.reshape(N, D)
    idx, wn = _route(flat, gate_w)

    sels, wsels = [], []
    for e in range(E):
        hit = idx == e                                         # [N, 2]
        sel = np.nonzero(hit.any(axis=1))[0]
        k = hit[sel, 1].astype(np.int64)                       # which top slot
        sels.append(sel)
        wsels.append(wn[sel, k])
    cmax = max(len(s) for s in sels)
    chunks = _chunk_plan(cmax)
    C = sum(chunks)

    offs = [sum(chunks[:i]) for i in range(len(chunks))]
    xT = np.ascontiguousarray(flat.T)                          # [D, N]

    def xg_layout(xgT, lo):
        # [D, C] -> [P, sum_t KD2*chunk_t]: per partition, chunk-major
        # blocks of [KD2, chunk] so each chunk DMA is contiguous.
        k0 = 0 if lo else KD2
        v = xgT.reshape(KD, P, C)[k0:k0 + KD2]                 # [KD2, P, C]
        blocks = [v[:, :, o:o + c].transpose(1, 0, 2).reshape(P, -1)
                  for o, c in zip(offs, chunks)]
        return np.ascontiguousarray(np.concatenate(blocks, axis=1))

    def w_layout(wT, nb):
        # [K, M] -> [nb, P, (K//P)*P]: per M-block, partition-major
        # stationary tile [P, K//P, 128] flattened (contiguous per
        # partition).
        K = wT.shape[0]
        kt = K // P
        v = wT.reshape(kt, P, nb, P).transpose(2, 1, 0, 3)     # [nb,P,kt,P]
        return np.ascontiguousarray(v.reshape(nb, P, kt * P))

    in_maps = []
    for e in range(E):
        sel = sels[e][:C]                  # tokens beyond C go to _host_ffn
        xgT = np.zeros((D, C), NP_BF16)
        xgT[:, :len(sel)] = xT[:, sel].astype(NP_BF16)
        scale_b = np.zeros((P, C), np.float32)
        scale_b[:, :len(sel)] = wsels[e][:C][None, :]
        in_maps.append({
            "xgLo": xg_layout(xgT, True),
            "xgHi": xg_layout(xgT, False),
            "w1L": w_layout(w1[e].T.astype(NP_BF16), HB),
            "w3L": w_layout(w3[e].T.astype(NP_BF16), HB),
            "w2L": w_layout(w2[e].T.astype(NP_BF16), DB),
            "scale_b": scale_b,
        })

    key = tuple(chunks)
    if key not in _program_cache:
        _program_cache[key] = _build_program(chunks)
    nc = _program_cache[key]

    res = run_bass_kernel_spmd(
        nc, in_maps, core_ids=list(range(E)),
        trace=TRACE,
        trace_cores=list(range(E)) if (TRACE and TRACE_ALL_CORES) else None,
    )
    LAST_RESULTS = res

    out = np.zeros((N, D), np.float32)
    for e in range(E):
        sel = sels[e][:C]
        out[sel] += res.results[e]["yT"][:, :len(sel)].T
        over = sels[e][C:]
        if len(over):
            out[over] += _host_ffn(flat[over], w1[e], w2[e], w3[e],
                                   wsels[e][C:])
    return out.reshape(B, T, D)


# revision 17
# speedup vs baseline: 1.0352x; 1.0352x over previous
"""MoE (top-2 of 8 experts, SwiGLU) Trainium2 kernel, expert-parallel over 8 cores.

Contract: kernel(**inputs) takes the FULL unsharded inputs
  x [2,2048,1024] f32, gate_w [8,1024] f32,
  w1 [8,2048,1024] f32, w2 [8,1024,2048] f32, w3 [8,2048,1024] f32
and returns the FULL output [2,2048,1024] f32.

Strategy (expert-parallel, per the hint "replicate the gate and all-to-all the
token dispatch"): routing (gate softmax + top-2) is computed on host; tokens
are dispatched (gathered) per expert; core e runs the SwiGLU FFN of expert e
over its ~N*TOPK/E assigned tokens (padded to capacity C), pre-scaled by the
combine weight; the host scatter-adds the two expert contributions per token.

Device kernel (per core, feature-major layout so no on-device transposes).
All matmul operands are bf16 (same 1 cycle/row PE rate as fp32r at 512-wide
moving dim, but half the DMA bytes, half the LDWEIGHTS time so weight loads
fully hide under the matmul stream, and half the SBUF footprint; fp32 PSUM
accumulate keeps rel err ~4e-3):
  h1T = w1 @ xg^T   [H, C]   (lhsT = w1T block, rhs = xgT)
  h3T = w3 @ xg^T   [H, C]
  aT  = silu(h1T) * h3T      (ACT Silu + DVE mul, PSUM->SBUF, bf16)
  yT  = (w2 @ aT) * combine  [D, C]  (DVE mul on PSUM eviction, f32 out)

Tokens are processed in free-dim chunks of <=512 (PSUM-bank bound). DMA is
spread over the three DGE queues (sync/scalar HWDGE + gpsimd SWDGE) so the
first chunk of xg and the first h-block's weights arrive in parallel, which
sets the time-to-first-matmul.
"""

import math
import sys

import numpy as np

for _p in ("/opt/trn_rl_repo", "/opt/pypackages"):
    if _p not in sys.path:
        sys.path.append(_p)

import ml_dtypes  # noqa: E402

import concourse.bass as bass  # noqa: E402
import concourse.tile as tile  # noqa: E402
from concourse import bacc, mybir  # noqa: E402
from concourse.bass_utils import run_bass_kernel_spmd  # noqa: E402

B, T, D, H, E, TOPK = 2, 2048, 1024, 2048, 8, 2
N = B * T
P = 128
KD = D // P   # 8  k-tiles over D
KD2 = KD // 2  # 4 k-tiles per xg half tile
KH = H // P   # 16 k-tiles over H
HB = H // P   # 16 h blocks of 128 (M dim, stage A)
DB = D // P   # 8  d blocks of 128 (M dim, stage B)

F32 = mybir.dt.float32
BF16 = mybir.dt.bfloat16
NP_BF16 = ml_dtypes.bfloat16

# set by test.py to capture an NTFF profile; kernel() stores results here
TRACE = False
TRACE_ALL_CORES = False
LAST_RESULTS = None

_program_cache = {}

# CoreSim doesn't implement Silu; simcheck.py overrides this to Sigmoid.
_ACT_FUNC = mybir.ActivationFunctionType.Silu


# Max tokens per expert handled on host when the count barely exceeds a
# 512 multiple (capacity-factor overflow): full 512-wide chunks minimize the
# per-matmul dispatch overhead.
OVERFLOW_MAX = 64


def _chunk_plan(cmax: int) -> list[int]:
    """Token-chunk sizes for the device capacity: each <=512 (PSUM bank),
    as equal as possible, 32-aligned, minimal total padding. If cmax is
    within OVERFLOW_MAX above a 512 multiple, use full 512 chunks and let
    the caller route the overflow tokens to the host FFN."""
    if cmax >= 512 and cmax - (cmax // 512) * 512 <= OVERFLOW_MAX:
        return [512] * (cmax // 512)
    n = max(1, math.ceil(cmax / 512))
    chunks = []
    rem = cmax
    for i in range(n):
        s = math.ceil(rem / (n - i) / 32) * 32
        s = min(max(s, 256), 512)
        chunks.append(s)
        rem -= s
    return chunks


def _host_ffn(x_rows, w1e, w2e, w3e, wts):
    """Host-side SwiGLU FFN for capacity-overflow tokens (<=64/expert),
    in bf16 operand precision to match the device kernel."""
    xb = x_rows.astype(NP_BF16).astype(np.float32)
    h1 = xb @ w1e.astype(NP_BF16).astype(np.float32).T
    h3 = xb @ w3e.astype(NP_BF16).astype(np.float32).T
    a = h1 / (1.0 + np.exp(-h1)) * h3
    a = a.astype(NP_BF16).astype(np.float32)
    return (a @ w2e.astype(NP_BF16).astype(np.float32).T) * wts[:, None]


def _build_program(chunks: list[int]):
    """Bass program for one core: expert FFN over C = sum(chunks) tokens."""
    C = sum(chunks)
    offs = [sum(chunks[:i]) for i in range(len(chunks))]
    tsls = [bass.ds(o, s) for o, s in zip(offs, chunks)]
    nt = len(chunks)

    nc = bacc.Bacc(
        "TRN2", target_bir_lowering=False, debug=False,
        enable_asserts=False, num_devices=8,
    )
    # All inputs are pre-tiled on host into partition-major layouts so every
    # DMA reads one big contiguous run per partition (2-8 KB descriptors).
    # Feature-split [D,C]-style layouts would give 256B-1KB runs in bf16,
    # which leaves the packet-rate-limited DMA engines at a fraction of the
    # ~360 GB/s per-core HBM bandwidth.
    #   xgLo/xgHi[p, t-block]: per chunk t a [KD2, chunk] block, flattened
    #   w1L/w3L[hb, p, :]: h-block hb's stationary tile [KD, P], flattened
    #   w2L[db, p, :]: d-block db's stationary tile [KH, P], flattened
    xgLo_d = nc.dram_tensor("xgLo", [P, KD2 * C], BF16,
                            kind="ExternalInput").ap()
    xgHi_d = nc.dram_tensor("xgHi", [P, KD2 * C], BF16,
                            kind="ExternalInput").ap()
    w1L_d = nc.dram_tensor("w1L", [HB, P, KD * P], BF16,
                           kind="ExternalInput").ap()
    w3L_d = nc.dram_tensor("w3L", [HB, P, KD * P], BF16,
                           kind="ExternalInput").ap()
    w2L_d = nc.dram_tensor("w2L", [DB, P, KH * P], BF16,
                           kind="ExternalInput").ap()
    scl_d = nc.dram_tensor("scale_b", [P, C], F32, kind="ExternalInput").ap()
    yT_d = nc.dram_tensor("yT", [D, C], F32, kind="ExternalOutput").ap()

    def xg_src(dram, t):
        return dram[:, KD2 * offs[t]:KD2 * (offs[t] + chunks[t])]

    with tile.TileContext(nc) as tc:
        with tc.tile_pool(name="resident", bufs=1) as res_pool, \
             tc.tile_pool(name="w13", bufs=3) as w13_pool, \
             tc.tile_pool(name="w2", bufs=3) as w2_pool, \
             tc.tile_pool(name="ev", bufs=3) as ev_pool, \
             tc.tile_pool(name="psum", bufs=2, space="PSUM") as ps_pool:

            # PE warm-up: the Tensor clock needs ~3 us of continuous busy to
            # reach 2.4 GHz (it idles at 0.65 GHz and a >0.1 us gap drops it
            # to 1.2 GHz). Dummy matmuls on a memset tile burn the initial
            # DMA wait so the first real matmul runs at full clock. The
            # dummies accumulate into one never-read psum tile on the "y"
            # tag (stage B's ring, unused during startup).
            warm = res_pool.tile([P, 3 * P], BF16, tag="warm")
            nc.vector.memset(warm[:], 0.0)
            warm_ps = ps_pool.tile([P, max(chunks)], F32, tag="y",
                                   name="warm_ps")
            NWARM = 20
            for i in range(NWARM):
                nc.tensor.matmul(warm_ps[:, 0:2 * P], warm[:, 0:P],
                                 warm[:, P:3 * P],
                                 start=(i == 0), stop=(i == NWARM - 1))

            # Startup critical path, ordered so each dependency lands
            # just-in-time for a gapless ramp (each tile is written by ONE
            # queue only; in-queue order = arrival order):
            #   sync   (HWDGE): w1t_h0, xg_lo c0, xg_lo c1, w3 h2.., w2
            #   scalar (HWDGE): xg_hi c0, w3t_h0, xg_hi c1, yT out
            #   gpsimd (SWDGE): w1t_h1, w3t_h1, w1 h2.., scl
            w1ts0, w3ts0 = [], []
            w1t = w13_pool.tile([P, KD, P], BF16, tag="w1_0",
                                bufs=2, name="w1t_0")
            nc.sync.dma_start(w1t[:], w1L_d[0])
            w1ts0.append(w1t)
            # xg SBUF tiles use the same flat chunk-major layout as their
            # DRAM images so each chunk DMA is one contiguous 4 KB run per
            # partition (src AND dst) -> single-descriptor transfers.
            xg_lo = res_pool.tile([P, KD2 * C], BF16, tag="xg_lo")
            xg_hi = res_pool.tile([P, KD2 * C], BF16, tag="xg_hi")

            def xg_slice(t):
                return bass.ds(KD2 * offs[t], KD2 * chunks[t])

            def rhs_lo(t, k):
                o = KD2 * offs[t] + k * chunks[t]
                return xg_lo[:, o:o + chunks[t]]

            def rhs_hi(t, k):
                o = KD2 * offs[t] + k * chunks[t]
                return xg_hi[:, o:o + chunks[t]]

            nc.sync.dma_start(xg_lo[:, xg_slice(0)], xg_src(xgLo_d, 0))
            nc.scalar.dma_start(xg_hi[:, xg_slice(0)], xg_src(xgHi_d, 0))
            w3t = w13_pool.tile([P, KD, P], BF16, tag="w3_0",
                                bufs=2, name="w3t_0")
            nc.scalar.dma_start(w3t[:], w3L_d[0])
            w3ts0.append(w3t)
            w1t = w13_pool.tile([P, KD, P], BF16, tag="w1_1",
                                bufs=2, name="w1t_1")
            nc.gpsimd.dma_start(w1t[:], w1L_d[1])
            w1ts0.append(w1t)
            w3t = w13_pool.tile([P, KD, P], BF16, tag="w3_1",
                                bufs=2, name="w3t_1")
            nc.gpsimd.dma_start(w3t[:], w3L_d[1])
            w3ts0.append(w3t)
            for t in range(1, nt):
                nc.sync.dma_start(xg_lo[:, xg_slice(t)], xg_src(xgLo_d, t))
                nc.scalar.dma_start(xg_hi[:, xg_slice(t)], xg_src(xgHi_d, t))
            act = res_pool.tile([P, KH, C], BF16, tag="act")

            # ---- stage A: act[H, C] = silu(w1 @ xgT) * (w3 @ xgT) ----
            # h-blocks processed in pairs with the token-chunk loop outside
            # the pair: two h-blocks of chunk-t compute run before chunk t+1
            # is touched, hiding the next xg chunk's DMA arrival. The first
            # pair's w1+w3 go on gpsimd (independent of the xg queues);
            # later pairs stream w1 on gpsimd and w3 on sync (in-order
            # behind xg, which has fully issued by then).
            for hp in range(0, HB, 2):
                pair = [h for h in (hp, hp + 1) if h < HB]
                if hp == 0:
                    w1ts = w1ts0
                    w3ts = w3ts0
                else:
                    w1ts, w3ts = [], []
                    for i, h in enumerate(pair):
                        w1t = w13_pool.tile([P, KD, P], BF16, tag=f"w1_{i}",
                                            bufs=2, name=f"w1t_{h}")
                        nc.gpsimd.dma_start(w1t[:], w1L_d[h])
                        w3t = w13_pool.tile([P, KD, P], BF16, tag=f"w3_{i}",
                                            bufs=2, name=f"w3t_{h}")
                        nc.sync.dma_start(w3t[:], w3L_d[h])
                        w1ts.append(w1t)
                        w3ts.append(w3t)
                for t in range(nt):
                    tsl = tsls[t]
                    for i, h in enumerate(pair):
                        ph1 = ps_pool.tile([P, chunks[t]], F32, tag="h1",
                                           bufs=3, name=f"ph1_{h}_{t}")
                        for k in range(KD2):
                            nc.tensor.matmul(ph1[:], w1ts[i][:, k, :],
                                             rhs_lo(t, k),
                                             start=(k == 0), stop=False)
                        for k in range(KD2):
                            nc.tensor.matmul(ph1[:], w1ts[i][:, KD2 + k, :],
                                             rhs_hi(t, k),
                                             start=False, stop=(k == KD2 - 1))
                        ph3 = ps_pool.tile([P, chunks[t]], F32, tag="h3",
                                           bufs=3, name=f"ph3_{h}_{t}")
                        for k in range(KD2):
                            nc.tensor.matmul(ph3[:], w3ts[i][:, k, :],
                                             rhs_lo(t, k),
                                             start=(k == 0), stop=False)
                        for k in range(KD2):
                            nc.tensor.matmul(ph3[:], w3ts[i][:, KD2 + k, :],
                                             rhs_hi(t, k),
                                             start=False, stop=(k == KD2 - 1))
                        asl = act[:, h, tsl]
                        nc.scalar.activation(asl, ph1[:], func=_ACT_FUNC)
                        nc.vector.tensor_mul(asl, asl, ph3[:])

            # combine-weight row (needed only for stage B evictions)
            scl = res_pool.tile([P, C], F32, tag="scl")
            nc.gpsimd.dma_start(scl[:], scl_d[:, :])

            # ---- stage B: yT[D, C] = (w2 @ act) * scale ----
            # The very last (d, t) group is split into two half-width psum
            # groups so the final evict+DMA chain after the last matmul
            # covers half the columns (the first half's eviction overlaps
            # the second half's matmuls).
            for d in range(DB):
                w2t = w2_pool.tile([P, KH, P], BF16, tag="w2")
                nc.sync.dma_start(w2t[:], w2L_d[d])
                for t in range(nt):
                    last = (d == DB - 1) and (t == nt - 1)
                    cw = chunks[t]
                    if last and cw % 2 == 0:
                        halves = [bass.ds(offs[t], cw // 2),
                                  bass.ds(offs[t] + cw // 2, cw // 2)]
                    else:
                        halves = [tsls[t]]
                    for hsl in halves:
                        hw = hsl.size
                        py = ps_pool.tile([P, hw], F32, tag="y")
                        for k in range(KH):
                            nc.tensor.matmul(py[:], w2t[:, k, :],
                                             act[:, k, hsl],
                                             start=(k == 0), stop=(k == KH - 1))
                        ysb = ev_pool.tile([P, hw], F32, tag="ysb")
                        nc.vector.tensor_mul(ysb[:], py[:], scl[:, hsl])
                        nc.scalar.dma_start(yT_d[d * P:(d + 1) * P, hsl],
                                            ysb[:])

    nc.compile()
    return nc


def _route(flat, gate_w):
    """Host replica of the reference router. Returns top-2 expert ids and
    combine weights (top-2 of softmax, renormalized)."""
    logits = flat @ gate_w.T                                   # [N, E] f32
    m = logits.max(axis=1, keepdims=True)
    p = np.exp((logits - m).astype(np.float32))
    probs = p / p.sum(axis=1, keepdims=True)
    idx = np.argsort(-probs, axis=1, kind="stable")[:, :TOPK]  # [N, 2]
    top = np.take_along_axis(probs, idx, axis=1)               # [N, 2]
    wn = top / top.sum(axis=1, keepdims=True)
    return idx, wn


def kernel(x, gate_w, w1, w2, w3):
    global LAST_RESULTS
    x = np.asarray(x, np.float32)
    gate_w = np.asarray(gate_w, np.float32)
    w1 = np.asarray(w1, np.float32)
    w2 = np.asarray(w2, np.float32)
    w3 = np.asarray(w3, np.float32)

    flat = x.reshape(N, D)
    idx, wn = _route(flat, gate_w)

    sels, wsels = [], []
    for e in range(E):
        hit = idx == e                                         # [N, 2]
        sel = np.nonzero(hit.any(axis=1))[0]
        k = hit[sel, 1].astype(np.int64)                       # which top slot
        sels.append(sel)
        wsels.append(wn[sel, k])
    cmax = max(len(s) for s in sels)
    chunks = _chunk_plan(cmax)
    C = sum(chunks)

    offs = [sum(chunks[:i]) for i in range(len(chunks))]
    xT = np.ascontiguousarray(flat.T)                          # [D, N]

    def xg_layout(xgT, lo):
        # [D, C] -> [P, sum_t KD2*chunk_t]: per partition, chunk-major
        # blocks of [KD2, chunk] so each chunk DMA is contiguous.
        k0 = 0 if lo else KD2
        v = xgT.reshape(KD, P, C)[k0:k0 + KD2]                 # [KD2, P, C]
        blocks = [v[:, :, o:o + c].transpose(1, 0, 2).reshape(P, -1)
                  for o, c in zip(offs, chunks)]
        return np.ascontiguousarray(np.concatenate(blocks, axis=1))

    def w_layout(wT, nb):
        # [K, M] -> [nb, P, (K//P)*P]: per M-block, partition-major
        # stationary tile [P, K//P, 128] flattened (contiguous per
        # partition).
        K = wT.shape[0]
        kt = K // P
        v = wT.reshape(kt, P, nb, P).transpose(2, 1, 0, 3)     # [nb,P,kt,P]
        return np.ascontiguousarray(v.reshape(nb, P, kt * P))

    in_maps = []
    for e in range(E):
        sel = sels[e][:C]                  # tokens beyond C go to _host_ffn
        xgT = np.zeros((D, C), NP_BF16)
        xgT[:, :len(sel)] = xT[:, sel].astype(NP_BF16)
        scale_b = np.zeros((P, C), np.float32)
        scale_b[:, :len(sel)] = wsels[e][:C][None, :]
        in_maps.append({
            "xgLo": xg_layout(xgT, True),
            "xgHi": xg_layout(xgT, False),
            "w1L": w_layout(w1[e].T.astype(NP_BF16), HB),
            "w3L": w_layout(w3[e].T.astype(NP_BF16), HB),
            "w2L": w_layout(w2[e].T.astype(NP_BF16), DB),
            "scale_b": scale_b,
        })

    key = tuple(chunks)
    if key not in _program_cache:
        _program_cache[key] = _build_program(chunks)
    nc = _program_cache[key]

    res = run_bass_kernel_spmd(
        nc, in_maps, core_ids=list(range(E)),
        trace=TRACE,
        trace_cores=list(range(E)) if (TRACE and TRACE_ALL_CORES) else None,
    )
    LAST_RESULTS = res

    out = np.zeros((N, D), np.float32)
    for e in range(E):
        sel = sels[e][:C]
        out[sel] += res.results[e]["yT"][:, :len(sel)].T
        over = sels[e][C:]
        if len(over):
            out[over] += _host_ffn(flat[over], w1[e], w2[e], w3[e],
                                   wsels[e][C:])
    return out.reshape(B, T, D)
